# revision 33
# baseline (speedup 1.0000x reference)
"""Trainium2 Bass kernel for nn_DictConv2d (FISTA convolutional sparse coding).

Reference computation (per sample):
    Wn  = W / ||W||_F per output filter          (128, 64, 3, 3)
    c_1 = relu(MU*conv(x, Wn) - thr)
    y_1 = c_1, t_1 = 1
    repeat 5x:
        r       = x - conv_T(y_k, Wn)
        c_{k+1} = relu(y_k + MU*conv(r, Wn) - thr)
        y_{k+1} = (1+mu_k) c_{k+1} - mu_k c_k        (mu_k from FISTA t-seq)
    return c_6

Rewritten in "u-form" so each iteration is pure matmul + tiny epilogues:
    b = MU*conv(x)            (computed once at init, fp32)
    z = y + b                 (fp32, split into bf16 hi+lo for injection)
    u = conv_T(y)             (9 col-tiled tap matmuls)
    c_{k+1} = relu(z - MU*conv(u) - thr)
            = relu( psum )    with -MU folded into the forward weights and
                              z_hi, z_lo injected via identity-matmul taps

The z hi/lo split matters: z feeds c directly (no MU damping), so a single
bf16 rounding of z costs ~8e-3 relative error; the two-tap split brings the
kernel back to the dictionary's own bf16 floor (~2.4e-3).

Mapping (pure data parallel over batch, 2 samples per core, 8 cores):

* All convs are 3x3 pad-1 on a 56x56 grid, done as "tap" matmuls gathering
  from zero-padded 58x58 images in SBUF, accumulating in PSUM. Row chunks
  of 8 output rows -> N=448 columns per forward matmul (one PSUM bank).

* conv_T (128ch -> 64ch): K=128, M=64 -> each tap runs as TWO CONCURRENT
  col-tiled matmuls (tile_position (0,0)/(0,64)): output rows h0..h0+3 into
  psum partitions 0-63, rows h0+4..h0+7 into partitions 64-127 (N=224 each).
  Measured 2x over an M=128 dup-weights form (98 vs 192 ns/tap).

* u is stored in a *stacked* padded layout (partitions 0-63 = upad,
  64-127 = upad shifted one row) so forward taps (0,dx)+(1,dx) fuse into
  K=128 matmuls. Eviction per chunk: 2 partition-aligned ACT copies
  (psum->SBUF bf16), 1 full ACT copy to a staging tile, and 2 DMAs that
  swap partition halves into the crossed destinations. The upad tile
  aliases the (dead after init) x-input tile to save SBUF.

* momentum: c-buffers store alpha_k * c_k with alpha_k = mu_k, giving
  y_{k+1} via ONE fused DVE scalar_tensor_tensor; z = y + b is one more
  DVE op; the bf16 derivations (ypad for conv_T, z_hi, z_lo) run on the
  otherwise-idle GpSimd engine.

Matmuls run in bf16 (1 PE cycle/row; fp32 would be 4x slower), accumulation
and the FISTA state stay fp32.
"""

import math
import sys

sys.path.insert(0, "/opt/trn_rl_repo")

import numpy as np
import ml_dtypes

import concourse.bass as bass
import concourse.tile as tile
from concourse import mybir
from concourse import bass_utils
from concourse.vector_clock import ScopedClock

F32 = mybir.dt.float32
BF16 = mybir.dt.bfloat16
AF = mybir.ActivationFunctionType
ALU = mybir.AluOpType

MU = 0.1
THR = MU * 0.1          # mu * lambda
N_ITERS = 5             # NUM_LAYERS - 1
H = W_ = 56
PH = H + 2              # 58
NPAD = PH * PH          # 3364
NPIX = H * W_           # 3136
NG = 8                  # output rows per chunk
NCHUNK = H // NG        # 7
NC_FREE = NG * W_       # 448 columns per forward matmul
HG = NG // 2            # rows per conv_T col-tile group
NCORES = 8
SPC = 2                 # samples per core


# ---------------------------------------------------------------------------
# FISTA momentum constants (exact, matches reference's t-sequence)
def _fista_consts():
    t = 1.0
    mu = []
    for _ in range(N_ITERS):
        t_next = (1.0 + math.sqrt(1.0 + 4.0 * t * t)) / 2.0
        mu.append((t - 1.0) / t_next)
        t = t_next
    # alpha[k] = scale stored in the c-buffer written by iteration k
    alpha = [mu[1], mu[2], mu[3], 1.0, 1.0]
    # s[k] = (1+mu_k)/alpha_{k+1} for the STT at iterations k=1..3
    s = [None, (1.0 + mu[1]) / alpha[1], (1.0 + mu[2]) / alpha[2],
         (1.0 + mu[3]) / alpha[3], None]
    inv_a0 = 1.0 / alpha[0]   # iteration 0's y-update: y = c_new / alpha[0]
    return mu, alpha, s, inv_a0


# ---------------------------------------------------------------------------
# Workaround: walrus in this container rejects >1 sync-wait per NoOp and >4
# on other opcodes. (a) TileContext's exit drain carries one wait per live
# proc semaphore -> split into single-wait SP NOPs. (b) a generic post-pass
# hoists extra waits from any scheduled instruction onto same-engine NOPs.
def _split_drain_and_barrier(self, tick_clock, wait_clock):
    nc = self.nc
    probe = nc.sync.nop()
    wait_clock.add_sem_waits(probe.ins, ScopedClock({None: tick_clock.global_clock}))
    ow = list(probe.ins.sync_info.on_wait) if probe.ins.sync_info else []
    probe.ins.sync_info = mybir.SyncInfo(on_wait=ow[:1], on_update=[])
    for w in ow[1:]:
        nop = nc.sync.nop()
        nop.ins.sync_info = mybir.SyncInfo(on_wait=[w], on_update=[])
    nc.sync.drain()
    nc.all_engine_barrier()
    assert self.sems is not None
    popped = nc._tile_sem_poison_stack.pop()
    assert popped is self._sem_poison
    nc.clear_and_free_semaphores(list(self.sems.allocated().values()))
    nc.all_engine_barrier()


tile.TileContext._drain_and_barrier = _split_drain_and_barrier

_WAIT_LIMIT = 1  # conservative: leave at most 1 wait on any instruction


def _hoist_excess_waits(nc):
    for fn in nc.m.functions:
        for blk in fn.blocks:
            insts = list(blk.instructions)
            out = []
            changed = False
            for inst in insts:
                si = inst.sync_info
                if si is not None and si.on_wait and len(si.on_wait) > _WAIT_LIMIT:
                    waits = list(si.on_wait)
                    keep = waits[-_WAIT_LIMIT:]
                    for w in waits[:-_WAIT_LIMIT]:
                        nop = mybir.InstNoOp(
                            name=nc.get_next_instruction_name(),
                            engine=inst.engine,
                            bass_nofuse=True,
                            sync_info=mybir.SyncInfo(on_wait=[w], on_update=[]),
                        )
                        nc.register_instruction(nop)
                        out.append(nop)
                    inst.sync_info = mybir.SyncInfo(
                        on_wait=keep, on_update=list(si.on_update)
                    )
                    changed = True
                out.append(inst)
            if changed:
                blk.instructions = out


# ---------------------------------------------------------------------------
def _build_program():
    mu, alpha, s_k, inv_a0 = _fista_consts()

    nc = bass.Bass("TRN2", debug=False, num_devices=NCORES)

    # ACT bias immediates resolve through a const-AP registry; register the
    # soft-threshold biases (then barrier so tile-scheduled readers can't
    # race the memsets).
    for v in {-THR} | {-a * THR for a in alpha}:
        t = nc.alloc_sbuf_tensor(f"const-f32-{v}", [128, 1], F32)
        nc.gpsimd.memset(t.ap(), v)
        nc.const_aps.aps[(F32, v)] = t.ap()
    nc.all_engine_barrier()

    xinb = nc.dram_tensor("xpadb", [SPC, 64, NPAD + PH], BF16, kind="ExternalInput")
    wct_d = nc.dram_tensor("wct", [128, 9 * 64], BF16, kind="ExternalInput")
    wfp_d = nc.dram_tensor("wfp", [128, 3 * 128], BF16, kind="ExternalInput")
    wfs_d = nc.dram_tensor("wfs", [128, 3 * 128], BF16, kind="ExternalInput")
    wfpn_d = nc.dram_tensor("wfpn", [128, 3 * 128], BF16, kind="ExternalInput")
    wfsn_d = nc.dram_tensor("wfsn", [128, 3 * 128], BF16, kind="ExternalInput")
    idn_d = nc.dram_tensor("idn", [128, 128], BF16, kind="ExternalInput")
    out_d = nc.dram_tensor("out", [SPC, 128, NPIX], F32, kind="ExternalOutput")

    with tile.TileContext(nc) as tc:
        with (
            tc.tile_pool(name="pers", bufs=1) as pers,
            tc.tile_pool(name="psum", bufs=3, space="PSUM") as psum,
        ):
            # ---- persistent SBUF state -------------------------------------
            wfp = pers.tile([128, 3 * 128], BF16, tag="wfp")
            wfs = pers.tile([128, 3 * 128], BF16, tag="wfs")
            idn = pers.tile([128, 128], BF16, tag="idn")
            wct = pers.tile([128, 9 * 64], BF16, tag="wct")
            wfpn = pers.tile([128, 3 * 128], BF16, tag="wfpn")
            wfsn = pers.tile([128, 3 * 128], BF16, tag="wfsn")
            # order DMAs by first use: fwd weights, then sample-0 x rows
            nc.sync.dma_start(out=wfp, in_=wfp_d.ap())
            nc.sync.dma_start(out=wfs, in_=wfs_d.ap())
            nc.sync.dma_start(out=idn, in_=idn_d.ap())

            xsb, ypad, zph, zpl, yf, bf32, cbuf = [], [], [], [], [], [], []
            for s in range(SPC):
                # xb doubles as upad after init (x is dead then; host-side
                # zero padding provides upad's zero borders)
                xb = pers.tile([128, NPAD], BF16, tag=f"xb_{s}", name=f"xb_{s}")
                yp = pers.tile([128, NPAD], BF16, tag=f"yp_{s}", name=f"yp_{s}")
                zh = pers.tile([128, NPAD], BF16, tag=f"zh_{s}", name=f"zh_{s}")
                zl = pers.tile([128, NPAD], BF16, tag=f"zl_{s}", name=f"zl_{s}")
                yfs = pers.tile([128, NPIX], F32, tag=f"yf_{s}", name=f"yf_{s}")
                bb = pers.tile([128, NPIX], F32, tag=f"bb_{s}", name=f"bb_{s}")
                ca = pers.tile([128, NPIX], F32, tag=f"ca_{s}", name=f"ca_{s}")
                cb = pers.tile([128, NPIX], F32, tag=f"cb_{s}", name=f"cb_{s}")
                xsb.append(xb); ypad.append(yp); zph.append(zh); zpl.append(zl)
                yf.append(yfs); bf32.append(bb); cbuf.append((ca, cb))
                # bf16 stacked-padded x, in row-bands so chunk 0 can start
                # while later rows stream in
                for b0, b1 in ((0, 14), (14, 30), (30, 58)):
                    f0, f1 = b0 * PH, b1 * PH
                    nc.sync.dma_start(out=xb[0:64, f0:f1],
                                      in_=xinb.ap()[s, :, f0:f1])
                    nc.sync.dma_start(out=xb[64:128, f0:f1],
                                      in_=xinb.ap()[s, :, PH + f0:PH + f1])
                nc.gpsimd.memset(yp, 0.0)
                nc.gpsimd.memset(zh, 0.0)
                nc.gpsimd.memset(zl, 0.0)
            zf = pers.tile([128, NPIX], F32, tag="zf", name="zf")  # shared
            nc.sync.dma_start(out=wct, in_=wct_d.ap())
            nc.sync.dma_start(out=wfpn, in_=wfpn_d.ap())
            nc.sync.dma_start(out=wfsn, in_=wfsn_d.ap())

            # 3-D views (partitions, padded-row, padded-col)
            u3 = [t.rearrange("p (r c) -> p r c", r=PH) for t in xsb]
            y3 = [t.rearrange("p (r c) -> p r c", r=PH) for t in ypad]
            zh3 = [t.rearrange("p (r c) -> p r c", r=PH) for t in zph]
            zl3 = [t.rearrange("p (r c) -> p r c", r=PH) for t in zpl]
            yfv = [t.rearrange("p (r c) -> p r c", c=W_) for t in yf]
            bfv = [t.rearrange("p (r c) -> p r c", c=W_) for t in bf32]
            zfv = zf.rearrange("p (r c) -> p r c", c=W_)

            def fwd_chunk(src3, g0, ptile, pair_w, sgl_w, z_taps, s):
                """forward-conv taps on a stacked padded source into ptile."""
                first = True
                for dx in range(3):  # pairs (0,dx)+(1,dx): K=128
                    nc.tensor.matmul(
                        ptile, pair_w[:, dx * 128:(dx + 1) * 128],
                        src3[:, g0:g0 + NG, dx:dx + W_],
                        start=first, stop=False)
                    first = False
                # singles (2,dx): zero-padded to K=128 (uniform K avoids a
                # ~210ns PE pipeline restart on every K transition)
                for dx in range(3):
                    last = (dx == 2) and not z_taps
                    nc.tensor.matmul(
                        ptile, sgl_w[:, dx * 128:(dx + 1) * 128],
                        src3[:, g0 + 2:g0 + 2 + NG, dx:dx + W_],
                        start=False, stop=last)
                if z_taps:  # += I @ z_hi + I @ z_lo
                    nc.tensor.matmul(
                        ptile, idn, zh3[s][:, g0 + 1:g0 + 1 + NG, 1:1 + W_],
                        start=False, stop=False)
                    nc.tensor.matmul(
                        ptile, idn, zl3[s][:, g0 + 1:g0 + 1 + NG, 1:1 + W_],
                        start=False, stop=True)

            def z_chain_a(s, g0):
                """Urgent per-chunk tail: ypad (bf16) unblocks next conv_T."""
                rows = slice(g0 + 1, g0 + 1 + NG)
                nc.vector.tensor_copy(
                    y3[s][:, rows, 1:1 + W_], yfv[s][:, g0:g0 + NG, :])

            def z_chain_b(s, g0):
                """Deferred per-chunk tail: z = y + b (fp32), z_hi (ACT),
                z_lo (DVE) — not needed until the NEXT fwd phase."""
                rows = slice(g0 + 1, g0 + 1 + NG)
                fl = slice(g0 * W_, (g0 + NG) * W_)
                nc.vector.tensor_tensor(
                    zf[:, fl], yf[s][:, fl], bf32[s][:, fl], ALU.add)
                nc.scalar.activation(
                    zh3[s][:, rows, 1:1 + W_], zfv[:, g0:g0 + NG, :], AF.Copy)
                nc.vector.tensor_tensor(
                    zl3[s][:, rows, 1:1 + W_], zfv[:, g0:g0 + NG, :],
                    zh3[s][:, rows, 1:1 + W_], ALU.subtract)

            CT_CHUNKS = [(0, 8), (16, 8), (32, 8), (48, 4)]  # (g0, half)

            def convt_taps(src3, g0, h, pc):
                """9 col-tiled conv_T taps over 2h output rows: rows
                g0..g0+h-1 -> psum partitions 0-63, g0+h..g0+2h-1 -> 64-127."""
                for t in range(9):
                    dy, dx = divmod(t, 3)
                    nc.tensor.matmul(
                        pc[0:64, :], wct[:, t * 64:(t + 1) * 64],
                        src3[:, g0 + dy:g0 + dy + h, dx:dx + W_],
                        start=(t == 0), stop=(t == 8), tile_position=(0, 0))
                    nc.tensor.matmul(
                        pc[64:128, :], wct[:, t * 64:(t + 1) * 64],
                        src3[:, g0 + h + dy:g0 + 2 * h + dy, dx:dx + W_],
                        start=(t == 0), stop=(t == 8), tile_position=(0, 64))

            # ---- init: b = MU conv(x); y_1 = relu(b - thr); z-chain --------
            for s in range(SPC):
                for c in range(NCHUNK):
                    g0 = c * NG
                    pf = psum.tile([128, NC_FREE], F32, tag="pf", name="pf")
                    fwd_chunk(u3[s], g0, pf, wfp, wfs, False, s)  # u3=x here
                    nc.scalar.activation(
                        bf32[s][:, g0 * W_:(g0 + NG) * W_], pf, AF.Copy)
                    nc.scalar.activation(
                        yf[s][:, g0 * W_:(g0 + NG) * W_], pf,
                        AF.Relu, bias=-THR, scale=1.0)
                    z_chain_a(s, g0)
                for c in range(NCHUNK):
                    z_chain_b(s, c * NG)

            # ---- 5 FISTA iterations ---------------------------------------
            def emit_convt(s):
                """u = conv_T(y) in 16-row chunks (8-row tail): psum[0:64] =
                u rows g0..g0+h-1, psum[64:128] = rows g0+h..g0+2h-1.
                Evict into the stacked upad layout: 2 aligned ACT copies +
                (full ACT copy to staging, 2 DMA partition swaps)."""
                for g0, h in CT_CHUNKS:
                    pc = psum.tile([128, 8 * W_], F32, tag="pc", name="pc",
                                   bufs=4)
                    pcs = pc[:, 0:h * W_]
                    convt_taps(y3[s], g0, h, pcs)
                    pc3 = pcs.rearrange("p (r c) -> p r c", c=W_)
                    # aligned halves
                    nc.scalar.activation(
                        u3[s][0:64, g0 + 1:g0 + 1 + h, 1:1 + W_],
                        pc3[0:64], AF.Copy)
                    nc.scalar.activation(
                        u3[s][64:128, g0 + h:g0 + 2 * h, 1:1 + W_],
                        pc3[64:128], AF.Copy)
                    # crossed halves: stage as bf16, DMA across partitions
                    stg = pers.tile([128, 8 * W_], BF16, tag="stg",
                                    name="stg", bufs=8)
                    sts = stg[:, 0:h * W_]
                    nc.scalar.activation(sts, pcs, AF.Copy)
                    stg3 = sts.rearrange("p (r c) -> p r c", c=W_)
                    nc.sync.dma_start(
                        out=u3[s][64:128, g0:g0 + h, 1:1 + W_],
                        in_=stg3[0:64])
                    nc.sync.dma_start(
                        out=u3[s][0:64, g0 + 1 + h:g0 + 1 + 2 * h, 1:1 + W_],
                        in_=stg3[64:128])

            for k in range(N_ITERS):
                cdst = [cbuf[s][k % 2] for s in range(SPC)]
                csrc = [cbuf[s][(k + 1) % 2] for s in range(SPC)]
                last = k == N_ITERS - 1
                # conv_T both samples (PE of sample 1 overlaps evictions of 0)
                emit_convt(0)
                emit_convt(1)
                # forward conv + shrink + momentum
                for s in range(SPC):
                    a = alpha[k]
                    for c in range(NCHUNK):
                        g0 = c * NG
                        fl = slice(g0 * W_, (g0 + NG) * W_)
                        pf = psum.tile([128, NC_FREE], F32, tag="pf", name="pf")
                        fwd_chunk(u3[s], g0, pf, wfpn, wfsn, True, s)
                        nc.scalar.activation(
                            cdst[s][:, fl], pf, AF.Relu,
                            bias=-a * THR, scale=a)
                        if last:  # stream c_6 out chunk-by-chunk
                            nc.sync.dma_start(
                                out=out_d.ap()[s, :, fl],
                                in_=cdst[s][:, fl])
                        else:
                            if k == 0:
                                nc.vector.tensor_scalar_mul(
                                    yf[s][:, fl], cdst[s][:, fl], inv_a0)
                            else:
                                nc.vector.scalar_tensor_tensor(
                                    yf[s][:, fl], cdst[s][:, fl], s_k[k],
                                    csrc[s][:, fl], ALU.mult, ALU.subtract)
                            z_chain_a(s, g0)
                    if not last:
                        for c in range(NCHUNK):
                            z_chain_b(s, c * NG)

    _hoist_excess_waits(nc)
    return nc


# ---------------------------------------------------------------------------
def _host_prep(x, W):
    """Precompute weight layouts + per-core padded inputs (all numpy)."""
    x = np.asarray(x, dtype=np.float32)
    W = np.asarray(W, dtype=np.float32)
    Wn = W / np.sqrt((W * W).sum(axis=(1, 2, 3), keepdims=True) + 1e-12)

    bf = ml_dtypes.bfloat16
    # conv_T lhsT per tap (dy,dx): [k=co(128), m=a(64)] = Wn[co, a, 2-dy, 2-dx]
    wct = np.empty((128, 9 * 64), dtype=np.float32)
    for t in range(9):
        dy, dx = divmod(t, 3)
        wct[:, t * 64:(t + 1) * 64] = Wn[:, :, 2 - dy, 2 - dx]
    # forward pair lhsT per dx: rows 0-63 = MU*Wn[:, :, 0, dx].T, 64-127 dy=1
    wfp = np.empty((128, 3 * 128), dtype=np.float32)
    wfs = np.zeros((128, 3 * 128), dtype=np.float32)  # rows 64-127 stay zero
    for dx in range(3):
        wfp[0:64, dx * 128:(dx + 1) * 128] = MU * Wn[:, :, 0, dx].T
        wfp[64:128, dx * 128:(dx + 1) * 128] = MU * Wn[:, :, 1, dx].T
        wfs[0:64, dx * 128:(dx + 1) * 128] = MU * Wn[:, :, 2, dx].T
    idn = np.eye(128, dtype=np.float32)

    n = x.shape[0]
    xpad = np.zeros((n, 64, PH, PH), dtype=np.float32)
    xpad[:, :, 1:1 + H, 1:1 + W_] = x
    xpad = xpad.reshape(n, 64, NPAD)
    xpad = np.concatenate(
        [xpad, np.zeros((n, 64, PH), dtype=np.float32)], axis=2)  # row of slack

    shared = {
        "wct": wct.astype(bf),
        "wfp": wfp.astype(bf),
        "wfs": wfs.astype(bf),
        "wfpn": (-wfp).astype(bf),
        "wfsn": (-wfs).astype(bf),
        "idn": idn.astype(bf),
    }
    xpadb = xpad.astype(bf)
    in_maps = []
    for core in range(NCORES):
        slb = xpadb[core * SPC:(core + 1) * SPC]
        in_maps.append({"xpadb": np.ascontiguousarray(slb), **shared})
    return in_maps


_CACHED_NC = None


def _get_nc():
    global _CACHED_NC
    if _CACHED_NC is None:
        _CACHED_NC = _build_program()
    return _CACHED_NC


def _run(x, W, **kwargs):
    in_maps = _host_prep(x, W)
    nc = _get_nc()
    res = bass_utils.run_bass_kernel_spmd(
        nc, in_maps, core_ids=list(range(NCORES)), **kwargs)
    outs = [res.results[i]["out"].reshape(SPC, 128, H, W_) for i in range(NCORES)]
    full = np.concatenate(outs, axis=0)
    return full, res


def kernel(x, W):
    out, _ = _run(x, W)
    return out


def kernel_profiled(x, W, tmpdir=None):
    _install_ntff_hook()
    out, res = _run(x, W, trace=True, tmpdir=tmpdir)
    return out, res


def _install_ntff_hook():
    """Register the axon NTFF profiling hook (the image's antenv lacks
    axon_hooks; drive the stable C ABI in libaxon_pjrt.so directly)."""
    import contextlib
    import ctypes
    import types

    try:
        from antenv.axon_hooks import get_axon_ntff_profile_hook  # noqa: F401
        return
    except ImportError:
        pass

    so_path = "/opt/axon/libaxon_pjrt.so"
    lib = ctypes.CDLL(so_path)
    if not hasattr(lib, "axon_start_nrt_profile"):
        return
    lib.axon_start_nrt_profile.argtypes = [
        ctypes.POINTER(ctypes.c_int64), ctypes.c_size_t]
    lib.axon_start_nrt_profile.restype = ctypes.c_int64
    lib.axon_stop_nrt_profile.argtypes = [ctypes.c_char_p]
    lib.axon_stop_nrt_profile.restype = ctypes.c_int64

    @contextlib.contextmanager
    def _hook(output_dir, device_ids):
        import jax
        jax.devices()
        if device_ids:
            ids = (ctypes.c_int64 * len(device_ids))(*device_ids)
            rc = lib.axon_start_nrt_profile(ids, len(device_ids))
        else:
            rc = lib.axon_start_nrt_profile(None, 0)
        if rc != 0:
            raise RuntimeError(f"axon_start_nrt_profile rc={rc}")
        try:
            yield
        finally:
            n = lib.axon_stop_nrt_profile(str(output_dir).encode())
            if n < 0:
                raise RuntimeError(f"axon_stop_nrt_profile rc={n}")
            if n == 0:
                print("WARNING: NTFF capture wrote no files")

    mod = types.ModuleType("antenv.axon_hooks")
    mod.get_axon_ntff_profile_hook = lambda: _hook
    mod.set_axon_ntff_profile_hook = lambda h: None
    sys.modules["antenv.axon_hooks"] = mod
